# revision 13
# baseline (speedup 1.0000x reference)
"""Causal multi-head self-attention on 8 Trainium2 NeuronCores.

Sharding: data-parallel over batch (B=2) x tensor-parallel over heads
(16 heads -> 4 per core).  Each core computes, for its batch element and
its 4 heads: Q/K/V projections, causal softmax attention, and a partial
output projection (row-parallel Wo).  The host sums the 4 TP partials per
batch and adds bo.

Layout trick: the host passes x.T and pre-transposed weight shards, so
every matmul contraction dim lands on SBUF partitions with no on-device
transposes anywhere:
  Q^T[o,t]   = sum_d WqT[d,o] * xT[d,t]        (scaled by 1/sqrt(hd))
  K^T                               likewise
  V[t,c]     = sum_d xT[d,t] * WvT[d,c]
  S^T[j,i]   = sum_c KT[c,j] * QT[c,i]         (per head; K zero-padded to
                                                128 - K=64 matmuls are ~1.7x
                                                slower on the PE)
  P^T        = exp(S^T) masked causally        (no max-subtraction needed:
                                                |scores| <= ~40 << fp32 range)
  ctx^T[c,i] = sum_j Vaug[j,c] * PT[j,i]       (Vaug has a ones column ->
                                                row 64 = softmax denominator)
  out[t,o]   = sum_c ctxT[c,t] * WoT[c,o]      (ctxT pre-divided by denom)

All matmul inputs are bf16 (1 cycle/row on the PE); PSUM accumulation and
the softmax-denominator path are fp32.  Matmuls of the same shape are
grouped back-to-back: consecutive shape changes cost ~1.7x on the PE.
"""
import math
import os

import ml_dtypes
import numpy as np

import concourse.bass as bass
import concourse.mybir as mybir
import concourse.tile as tile
from concourse import bacc
from concourse.bass_utils import run_bass_kernel_spmd

F32 = mybir.dt.float32
BF16 = mybir.dt.bfloat16
AF = mybir.ActivationFunctionType
OP = mybir.AluOpType

B, T, D, H, HD = 2, 2048, 1024, 16, 64
NCORES, TP = 8, 4
HPC = H // TP          # heads per core = 4
CS = HPC * HD          # channel shard per core = 256
SCALE = 1.0 / math.sqrt(HD)
KB = D // 128          # 8 k-blocks of the d contraction
TCH = 512              # i-chunk (queries per attention inner pass)
NI = T // TCH          # 4 i-chunks
NTB = T // 128         # 16 token blocks
JGRP = 4               # j-tiles per same-shape matmul burst

_CACHE: dict = {}
LAST_EXEC_NS = None
LAST_RESULTS = None


def _build(has_bias: bool):
    nc = bacc.Bacc("TRN2", target_bir_lowering=False, debug=False,
                   num_devices=NCORES)

    xT_d = nc.dram_tensor("xT", [D, T], BF16, kind="ExternalInput").ap()
    wqT_d = nc.dram_tensor("wqT", [D, CS], BF16, kind="ExternalInput").ap()
    wkT_d = nc.dram_tensor("wkT", [D, CS], BF16, kind="ExternalInput").ap()
    wvT_d = nc.dram_tensor("wvT", [D, CS], BF16, kind="ExternalInput").ap()
    woT_d = nc.dram_tensor("woT", [CS, D], BF16, kind="ExternalInput").ap()
    bq_d = nc.dram_tensor("bq", [1, CS], BF16, kind="ExternalInput").ap()
    bk_d = nc.dram_tensor("bk", [1, CS], BF16, kind="ExternalInput").ap()
    bv_d = nc.dram_tensor("bv", [1, CS], BF16, kind="ExternalInput").ap()
    ones_d = nc.dram_tensor("ones", [1, T], BF16, kind="ExternalInput").ap()
    onesc_d = nc.dram_tensor("onesc", [128, HPC], BF16, kind="ExternalInput").ap()
    out_d = nc.dram_tensor("out", [T, D], F32, kind="ExternalOutput").ap()

    with tile.TileContext(nc) as tc:
        with (
            tc.tile_pool(name="persist", bufs=1) as pp,
        ):
            # persistent across phases; QT/KT hold head h in block h rows
            # 0-63, rows 64-127 are zeros so the S^T matmul can run K=128.
            QT = pp.tile([128, HPC, T], BF16)
            KT = pp.tile([128, HPC, T], BF16)
            # V with a ones column per head: head h at cols 65h..65h+64
            V = pp.tile([128, NTB, HPC * (HD + 1)], BF16)
            CT = pp.tile([128, 2, T], BF16)      # ctx^T (normalized)
            WO = pp.tile([128, 2, D], BF16)
            ONES = pp.tile([1, T], BF16)

            nc.gpsimd.memset(QT[:], 0.0)
            nc.gpsimd.memset(KT[:], 0.0)
            nc.sync.dma_start(out=ONES[:], in_=ones_d[:])
            nc.sync.dma_start(out=WO[:], in_=woT_d.rearrange("(a p) o -> p a o", p=128))

            # ---------- phase 1: projections ----------
            with (
                tc.tile_pool(name="ph1", bufs=1) as p1,
                tc.tile_pool(name="ps1", bufs=4, space="PSUM") as ps1,
            ):
                XT = p1.tile([128, KB, T], BF16)
                WQ = p1.tile([128, KB, CS], BF16)
                WK = p1.tile([128, KB, CS], BF16)
                WV = p1.tile([128, KB, CS], BF16)
                OC = p1.tile([128, HPC], BF16)

                nc.sync.dma_start(out=WQ[:], in_=wqT_d.rearrange("(a p) c -> p a c", p=128))
                nc.sync.dma_start(out=WK[:], in_=wkT_d.rearrange("(a p) c -> p a c", p=128))
                nc.scalar.dma_start(out=WV[:], in_=wvT_d.rearrange("(a p) c -> p a c", p=128))
                nc.scalar.dma_start(out=OC[:], in_=onesc_d[:])
                if has_bias:
                    BQ = p1.tile([1, CS], BF16)
                    BK = p1.tile([1, CS], BF16)
                    BV = p1.tile([1, CS], BF16)
                    nc.scalar.dma_start(out=BQ[:], in_=bq_d[:])
                    nc.scalar.dma_start(out=BK[:], in_=bk_d[:])
                    nc.scalar.dma_start(out=BV[:], in_=bv_d[:])
                # x: split across both HWDGE queues for faster arrival
                xt_view = xT_d.rearrange("(a p) t -> a p t", p=128)
                for kb in range(KB):
                    eng = nc.sync if kb % 2 == 0 else nc.scalar
                    eng.dma_start(out=XT[:, kb, :], in_=xt_view[kb])

                # Q^T and K^T: psum[o-block(2 heads), t-chunk] -> per-head
                # blocks of QT/KT (rows 0-63), scaled on the DVE copy.
                for W_sb, bname, dst, scl in (
                        (WQ, "bq", QT, SCALE), (WK, "bk", KT, 1.0)):
                    for ob in range(2):
                        for tcn in range(NI):
                            tsl = slice(tcn * TCH, (tcn + 1) * TCH)
                            p = ps1.tile([128, TCH], F32, tag="ps1")
                            for kb in range(KB):
                                nc.tensor.matmul(
                                    p[:],
                                    W_sb[:, kb, ob * 128:(ob + 1) * 128],
                                    XT[:, kb, tsl],
                                    start=(kb == 0),
                                    stop=(kb == KB - 1 and not has_bias))
                            if has_bias:
                                bt = {"bq": BQ, "bk": BK}[bname]
                                nc.tensor.matmul(
                                    p[:], bt[0:1, ob * 128:(ob + 1) * 128],
                                    ONES[0:1, tsl], start=False, stop=True)
                            for hh in range(2):
                                h = 2 * ob + hh
                                if scl == 1.0:
                                    nc.vector.tensor_copy(
                                        out=dst[0:64, h, tsl],
                                        in_=p[64 * hh:64 * hh + 64, :])
                                else:
                                    nc.vector.tensor_scalar_mul(
                                        out=dst[0:64, h, tsl],
                                        in0=p[64 * hh:64 * hh + 64, :],
                                        scalar1=scl)

                # V: out[t-block, c] then scatter into the 65-col head slots
                v_view = V.rearrange("p n (h c) -> p n h c", c=HD + 1)
                for tb in range(NTB):
                    p = ps1.tile([128, TCH], F32, tag="ps1")
                    for kb in range(KB):
                        nc.tensor.matmul(
                            p[:, 0:CS],
                            XT[:, kb, tb * 128:(tb + 1) * 128],
                            WV[:, kb, :],
                            start=(kb == 0),
                            stop=(kb == KB - 1 and not has_bias))
                    if has_bias:
                        nc.tensor.matmul(
                            p[:, 0:CS], ONES[0:1, tb * 128:(tb + 1) * 128],
                            BV[0:1, :], start=False, stop=True)
                    nc.vector.tensor_copy(
                        out=v_view[:, tb, :, 0:HD],
                        in_=p[:, 0:CS].rearrange("p (h c) -> p h c", c=HD))
                    nc.vector.tensor_copy(
                        out=v_view[:, tb, :, HD:HD + 1],
                        in_=OC[:].unsqueeze(-1))

            # ---------- phase 2+3: attention + output projection ----------
            with (
                tc.tile_pool(name="pt", bufs=6) as ptp,
                tc.tile_pool(name="sm", bufs=5) as smp,
                tc.tile_pool(name="ost", bufs=8) as ostp,
                tc.tile_pool(name="pss", bufs=2, space="PSUM") as pss,
                tc.tile_pool(name="psc", bufs=4, space="PSUM") as psc,
                tc.tile_pool(name="pso", bufs=2, space="PSUM") as pso,
            ):
                for icn in range(NI):
                    jt_max = (icn + 1) * (TCH // 128)
                    isl = slice(icn * TCH, (icn + 1) * TCH)
                    rcbs, pctxs = [], []
                    for h in range(HPC):
                        cb, po = h // 2, 64 * (h % 2)   # CT block/row-offset
                        pctx = psc.tile([128, TCH], F32, tag="psc",
                                        name=f"pctx{icn}_{h}")
                        # j-tiles in bursts of JGRP: S x4 back-to-back, then
                        # exp x4, then PV x4 - same-shape matmul runs are
                        # ~1.7x faster than alternating shapes.
                        for j0 in range(0, jt_max, JGRP):
                            jn = min(JGRP, jt_max - j0)
                            pts = []
                            pss_t = []
                            for jj in range(jn):
                                jt = j0 + jj
                                ps = pss.tile([128, TCH], F32, tag="pss",
                                              name=f"ps{icn}_{h}_{jt}")
                                nc.tensor.matmul(
                                    ps[:],
                                    KT[:, h, jt * 128:(jt + 1) * 128],
                                    QT[:, h, isl],
                                    start=True, stop=True)
                                pss_t.append(ps)
                            for jj in range(jn):
                                jt = j0 + jj
                                pt = ptp.tile([128, TCH], BF16, tag="pt",
                                              name=f"pt{icn}_{h}_{jt}")
                                nc.scalar.activation(pt[:], pss_t[jj][:], AF.Exp)
                                if jt >= icn * (TCH // 128):
                                    # diagonal block: keep where global i >= j
                                    nc.gpsimd.affine_select(
                                        out=pt[:], in_=pt[:],
                                        compare_op=OP.is_ge, fill=0.0,
                                        base=icn * TCH - jt * 128,
                                        channel_multiplier=-1,
                                        pattern=[[1, TCH]])
                                pts.append(pt)
                            for jj in range(jn):
                                jt = j0 + jj
                                nc.tensor.matmul(
                                    pctx[0:HD + 1, :],
                                    v_view[:, jt, h, :],
                                    pts[jj][:],
                                    start=(jt == 0), stop=(jt == jt_max - 1))
                        # normalize part 1 (per head, DVE only): 1/denom
                        dnr = smp.tile([1, TCH], F32, tag="dnr")
                        nc.vector.tensor_copy(out=dnr[:], in_=pctx[HD:HD + 1, :])
                        rcb = smp.tile([1, TCH], BF16, tag="rcb",
                                       name=f"rcb{icn}_{h}")
                        with nc.allow_low_precision(reason="softmax denom"):
                            rc = smp.tile([1, TCH], F32, tag="rc")
                            nc.vector.reciprocal_approx_fast(out=rc[:], in_=dnr[:])
                        nc.vector.tensor_copy(out=rcb[:], in_=rc[:])
                        rcbs.append(rcb)
                        pctxs.append(pctx)

                    # normalize part 2: batched K=1 ones-matmul broadcasts
                    # (batching avoids PE shape switches inside the S/PV
                    # streams; gpsimd partition_broadcast would serialize
                    # against in-flight DMA queues)
                    bcs = []
                    for h in range(HPC):
                        pb = pso.tile([128, TCH], F32, tag="pso",
                                      name=f"pb{icn}_{h}")
                        nc.tensor.matmul(pb[0:64, :], ONES[0:1, 0:64],
                                         rcbs[h][0:1, :], start=True, stop=True)
                        bc = smp.tile([64, TCH], F32, tag="bc",
                                      name=f"bc{icn}_{h}")
                        nc.scalar.copy(out=bc[:], in_=pb[0:64, :])
                        bcs.append(bc)
                    for h in range(HPC):
                        cb, po = h // 2, 64 * (h % 2)
                        nc.vector.tensor_tensor(
                            out=CT[po:po + 64, cb, isl],
                            in0=pctxs[h][0:HD, :], in1=bcs[h][:],
                            op=OP.mult)

                    # output projection for this i-chunk's token blocks
                    for tb in range(icn * 4, icn * 4 + 4):
                        for on in range(2):
                            p = pso.tile([128, TCH], F32, tag="pso")
                            for cbk in range(2):
                                nc.tensor.matmul(
                                    p[:],
                                    CT[:, cbk, tb * 128:(tb + 1) * 128],
                                    WO[:, cbk, on * TCH:(on + 1) * TCH],
                                    start=(cbk == 0), stop=(cbk == 1))
                            ob_sb = ostp.tile([128, TCH], F32, tag="ost")
                            nc.vector.tensor_copy(out=ob_sb[:], in_=p[:])
                            nc.sync.dma_start(
                                out=out_d[tb * 128:(tb + 1) * 128,
                                          on * TCH:(on + 1) * TCH],
                                in_=ob_sb[:])

    nc.compile()
    return nc


def _get_nc(has_bias: bool):
    key = ("nc", has_bias)
    if key not in _CACHE:
        _CACHE[key] = _build(has_bias)
    return _CACHE[key]


def _maybe_wire_ntff_hook():
    try:
        import antenv.axon_hooks  # noqa: F401  already present
        return
    except ImportError:
        pass
    try:
        import sys, types
        import trn_agent_boot.trn_boot as boot
        hook = boot._ntff_profile_via_ctypes("/opt/axon/libaxon_pjrt.so")
        mod = types.ModuleType("antenv.axon_hooks")
        mod.get_axon_ntff_profile_hook = lambda: hook
        mod.set_axon_ntff_profile_hook = lambda h: None
        sys.modules["antenv.axon_hooks"] = mod
    except Exception:
        pass


def kernel(x, Wq, bq, Wk, bk, Wv, bv, Wo, bo, _trace=False):
    global LAST_EXEC_NS, LAST_RESULTS
    x = np.asarray(x, np.float32)
    Wq = np.asarray(Wq, np.float32); bq = np.asarray(bq, np.float32)
    Wk = np.asarray(Wk, np.float32); bk = np.asarray(bk, np.float32)
    Wv = np.asarray(Wv, np.float32); bv = np.asarray(bv, np.float32)
    Wo = np.asarray(Wo, np.float32); bo = np.asarray(bo, np.float32)

    has_bias = bool(np.any(bq) or np.any(bk) or np.any(bv))
    nc = _get_nc(has_bias)

    BFNP = ml_dtypes.bfloat16
    ones = np.ones((1, T), BFNP)
    onesc = np.ones((128, HPC), BFNP)
    xTs = [np.ascontiguousarray(x[b].T).astype(BFNP) for b in range(B)]

    in_maps = []
    for c in range(NCORES):
        b, tpr = divmod(c, TP)
        rows = slice(CS * tpr, CS * (tpr + 1))
        in_maps.append({
            "xT": xTs[b],
            "wqT": np.ascontiguousarray(Wq[rows, :].T).astype(BFNP),
            "wkT": np.ascontiguousarray(Wk[rows, :].T).astype(BFNP),
            "wvT": np.ascontiguousarray(Wv[rows, :].T).astype(BFNP),
            "woT": np.ascontiguousarray(Wo[:, rows].T).astype(BFNP),
            "bq": np.ascontiguousarray(bq[rows]).reshape(1, CS).astype(BFNP),
            "bk": np.ascontiguousarray(bk[rows]).reshape(1, CS).astype(BFNP),
            "bv": np.ascontiguousarray(bv[rows]).reshape(1, CS).astype(BFNP),
            "ones": ones,
            "onesc": onesc,
        })

    if _trace:
        _maybe_wire_ntff_hook()
    res = run_bass_kernel_spmd(nc, in_maps, core_ids=list(range(NCORES)),
                               trace=bool(_trace))
    LAST_EXEC_NS = res.exec_time_ns
    LAST_RESULTS = res

    out = np.empty((B, T, D), np.float32)
    for b in range(B):
        acc = res.results[TP * b]["out"].astype(np.float32)
        for tpr in range(1, TP):
            acc = acc + res.results[TP * b + tpr]["out"]
        out[b] = acc + bo[None, :]
    return out


# revision 14
# speedup vs baseline: 1.1004x; 1.1004x over previous
"""Causal multi-head self-attention on 8 Trainium2 NeuronCores.

Sharding: data-parallel over batch (B=2) x tensor-parallel over heads
(16 heads -> 4 per core).  Each core computes, for its batch element and
its 4 heads: Q/K/V projections, causal softmax attention, and a partial
output projection (row-parallel Wo).  The host sums the 4 TP partials per
batch and adds bo.

Layout trick: the host passes x.T and pre-transposed weight shards, so
every matmul contraction dim lands on SBUF partitions with no on-device
transposes anywhere:
  Q^T[o,t]   = sum_d WqT[d,o] * xT[d,t]        (scaled by 1/sqrt(hd))
  K^T                               likewise
  V[t,c]     = sum_d xT[d,t] * WvT[d,c]
  S^T[j,i]   = sum_c KT[c,j] * QT[c,i]         (per head; K zero-padded to
                                                128 - K=64 matmuls are ~1.7x
                                                slower on the PE)
  P^T        = exp(S^T) masked causally        (no max-subtraction needed:
                                                |scores| <= ~40 << fp32 range)
  ctx^T[c,i] = sum_j Vaug[j,c] * PT[j,i]       (Vaug has a ones column ->
                                                row 64 = softmax denominator)
  out[t,o]   = sum_c ctxT[c,t] * WoT[c,o]      (ctxT pre-divided by denom)

All matmul inputs are bf16 (1 cycle/row on the PE); PSUM accumulation and
the softmax-denominator path are fp32.  Matmuls of the same shape are
grouped back-to-back: consecutive shape changes cost ~1.7x on the PE.
"""
import math
import os

import ml_dtypes
import numpy as np

import concourse.bass as bass
import concourse.mybir as mybir
import concourse.tile as tile
from concourse import bacc
from concourse.bass_utils import run_bass_kernel_spmd

F32 = mybir.dt.float32
BF16 = mybir.dt.bfloat16
AF = mybir.ActivationFunctionType
OP = mybir.AluOpType

B, T, D, H, HD = 2, 2048, 1024, 16, 64
NCORES, TP = 8, 4
HPC = H // TP          # heads per core = 4
CS = HPC * HD          # channel shard per core = 256
SCALE = 1.0 / math.sqrt(HD)
KB = D // 128          # 8 k-blocks of the d contraction
TCH = 512              # i-chunk (queries per attention inner pass)
NI = T // TCH          # 4 i-chunks
NTB = T // 128         # 16 token blocks
JGRP = 4               # j-tiles per same-shape matmul burst

_CACHE: dict = {}
LAST_EXEC_NS = None
LAST_RESULTS = None


def _build(has_bias: bool):
    nc = bacc.Bacc("TRN2", target_bir_lowering=False, debug=False,
                   num_devices=NCORES)

    xT_d = nc.dram_tensor("xT", [D, T], BF16, kind="ExternalInput").ap()
    wqT_d = nc.dram_tensor("wqT", [D, CS], BF16, kind="ExternalInput").ap()
    wkT_d = nc.dram_tensor("wkT", [D, CS], BF16, kind="ExternalInput").ap()
    wvT_d = nc.dram_tensor("wvT", [D, CS], BF16, kind="ExternalInput").ap()
    woT_d = nc.dram_tensor("woT", [CS, D], BF16, kind="ExternalInput").ap()
    bq_d = nc.dram_tensor("bq", [1, CS], BF16, kind="ExternalInput").ap()
    bk_d = nc.dram_tensor("bk", [1, CS], BF16, kind="ExternalInput").ap()
    bv_d = nc.dram_tensor("bv", [1, CS], BF16, kind="ExternalInput").ap()
    ones_d = nc.dram_tensor("ones", [1, T], BF16, kind="ExternalInput").ap()
    onesc_d = nc.dram_tensor("onesc", [128, HPC], BF16, kind="ExternalInput").ap()
    out_d = nc.dram_tensor("out", [T, D], F32, kind="ExternalOutput").ap()

    with tile.TileContext(nc) as tc:
        with (
            tc.tile_pool(name="persist", bufs=1) as pp,
        ):
            # persistent across phases; QT/KT hold head h in block h rows
            # 0-63, rows 64-127 are zeros so the S^T matmul can run K=128.
            QT = pp.tile([128, HPC, T], BF16)
            KT = pp.tile([128, HPC, T], BF16)
            # V with a ones column per head: head h at cols 65h..65h+64
            V = pp.tile([128, NTB, HPC * (HD + 1)], BF16)
            CT = pp.tile([128, 2, T], BF16)      # ctx^T (normalized)
            WO = pp.tile([128, 2, D], BF16)
            ONES = pp.tile([1, T], BF16)

            nc.gpsimd.memset(QT[:], 0.0)
            nc.gpsimd.memset(KT[:], 0.0)
            nc.sync.dma_start(out=ONES[:], in_=ones_d[:])
            nc.sync.dma_start(out=WO[:], in_=woT_d.rearrange("(a p) o -> p a o", p=128))

            # ---------- phase 1: projections ----------
            with (
                tc.tile_pool(name="ph1", bufs=1) as p1,
                tc.tile_pool(name="ps1", bufs=4, space="PSUM") as ps1,
            ):
                XT = p1.tile([128, KB, T], BF16)
                WQ = p1.tile([128, KB, CS], BF16)
                WK = p1.tile([128, KB, CS], BF16)
                WV = p1.tile([128, KB, CS], BF16)
                OC = p1.tile([128, HPC], BF16)

                nc.sync.dma_start(out=WQ[:], in_=wqT_d.rearrange("(a p) c -> p a c", p=128))
                nc.sync.dma_start(out=WK[:], in_=wkT_d.rearrange("(a p) c -> p a c", p=128))
                nc.scalar.dma_start(out=WV[:], in_=wvT_d.rearrange("(a p) c -> p a c", p=128))
                nc.scalar.dma_start(out=OC[:], in_=onesc_d[:])
                if has_bias:
                    BQ = p1.tile([1, CS], BF16)
                    BK = p1.tile([1, CS], BF16)
                    BV = p1.tile([1, CS], BF16)
                    nc.scalar.dma_start(out=BQ[:], in_=bq_d[:])
                    nc.scalar.dma_start(out=BK[:], in_=bk_d[:])
                    nc.scalar.dma_start(out=BV[:], in_=bv_d[:])
                # x: split across both HWDGE queues for faster arrival
                xt_view = xT_d.rearrange("(a p) t -> a p t", p=128)
                for kb in range(KB):
                    eng = nc.sync if kb % 2 == 0 else nc.scalar
                    eng.dma_start(out=XT[:, kb, :], in_=xt_view[kb])

                # Q^T and K^T: psum[o-block(2 heads), t-chunk] -> per-head
                # blocks of QT/KT (rows 0-63), scaled on the DVE copy.
                for W_sb, bname, dst, scl in (
                        (WQ, "bq", QT, SCALE), (WK, "bk", KT, 1.0)):
                    for ob in range(2):
                        for tcn in range(NI):
                            tsl = slice(tcn * TCH, (tcn + 1) * TCH)
                            p = ps1.tile([128, TCH], F32, tag="ps1")
                            for kb in range(KB):
                                nc.tensor.matmul(
                                    p[:],
                                    W_sb[:, kb, ob * 128:(ob + 1) * 128],
                                    XT[:, kb, tsl],
                                    start=(kb == 0),
                                    stop=(kb == KB - 1 and not has_bias))
                            if has_bias:
                                bt = {"bq": BQ, "bk": BK}[bname]
                                nc.tensor.matmul(
                                    p[:], bt[0:1, ob * 128:(ob + 1) * 128],
                                    ONES[0:1, tsl], start=False, stop=True)
                            for hh in range(2):
                                h = 2 * ob + hh
                                if scl == 1.0:
                                    nc.vector.tensor_copy(
                                        out=dst[0:64, h, tsl],
                                        in_=p[64 * hh:64 * hh + 64, :])
                                else:
                                    nc.vector.tensor_scalar_mul(
                                        out=dst[0:64, h, tsl],
                                        in0=p[64 * hh:64 * hh + 64, :],
                                        scalar1=scl)

                # V: out[t-block, c] then scatter into the 65-col head slots
                v_view = V.rearrange("p n (h c) -> p n h c", c=HD + 1)
                for tb in range(NTB):
                    p = ps1.tile([128, TCH], F32, tag="ps1")
                    for kb in range(KB):
                        nc.tensor.matmul(
                            p[:, 0:CS],
                            XT[:, kb, tb * 128:(tb + 1) * 128],
                            WV[:, kb, :],
                            start=(kb == 0),
                            stop=(kb == KB - 1 and not has_bias))
                    if has_bias:
                        nc.tensor.matmul(
                            p[:, 0:CS], ONES[0:1, tb * 128:(tb + 1) * 128],
                            BV[0:1, :], start=False, stop=True)
                    nc.vector.tensor_copy(
                        out=v_view[:, tb, :, 0:HD],
                        in_=p[:, 0:CS].rearrange("p (h c) -> p h c", c=HD))
                    nc.vector.tensor_copy(
                        out=v_view[:, tb, :, HD:HD + 1],
                        in_=OC[:].unsqueeze(-1))

            # ---------- phase 2+3: attention + output projection ----------
            with (
                tc.tile_pool(name="pt", bufs=6) as ptp,
                tc.tile_pool(name="sm", bufs=3) as smp,
                tc.tile_pool(name="ost", bufs=8) as ostp,
                tc.tile_pool(name="pss", bufs=3, space="PSUM") as pss,
                tc.tile_pool(name="psc", bufs=3, space="PSUM") as psc,
                tc.tile_pool(name="pso", bufs=2, space="PSUM") as pso,
            ):
                for icn in range(NI):
                    jt_max = (icn + 1) * (TCH // 128)
                    isl = slice(icn * TCH, (icn + 1) * TCH)
                    for h in range(HPC):
                        cb, po = h // 2, 64 * (h % 2)   # CT block/row-offset
                        pctx = psc.tile([128, TCH], F32, tag="psc",
                                        name=f"pctx{icn}_{h}")
                        # j-tiles in bursts of JGRP: S x4 back-to-back, then
                        # exp x4, then PV x4 - same-shape matmul runs are
                        # ~1.7x faster than alternating shapes.
                        for j0 in range(0, jt_max, JGRP):
                            jn = min(JGRP, jt_max - j0)
                            pts = []
                            pss_t = []
                            for jj in range(jn):
                                jt = j0 + jj
                                ps = pss.tile([128, TCH], F32, tag="pss",
                                              name=f"ps{icn}_{h}_{jt}")
                                nc.tensor.matmul(
                                    ps[:],
                                    KT[:, h, jt * 128:(jt + 1) * 128],
                                    QT[:, h, isl],
                                    start=True, stop=True)
                                pss_t.append(ps)
                            for jj in range(jn):
                                jt = j0 + jj
                                pt = ptp.tile([128, TCH], BF16, tag="pt",
                                              name=f"pt{icn}_{h}_{jt}")
                                nc.scalar.activation(pt[:], pss_t[jj][:], AF.Exp)
                                if jt >= icn * (TCH // 128):
                                    # diagonal block: keep where global i >= j
                                    nc.gpsimd.affine_select(
                                        out=pt[:], in_=pt[:],
                                        compare_op=OP.is_ge, fill=0.0,
                                        base=icn * TCH - jt * 128,
                                        channel_multiplier=-1,
                                        pattern=[[1, TCH]])
                                pts.append(pt)
                            for jj in range(jn):
                                jt = j0 + jj
                                nc.tensor.matmul(
                                    pctx[0:HD + 1, :],
                                    v_view[:, jt, h, :],
                                    pts[jj][:],
                                    start=(jt == 0), stop=(jt == jt_max - 1))
                        # normalize: 1/denom broadcast over the 64 ctx rows
                        dnr = smp.tile([1, TCH], F32, tag="dnr")
                        nc.vector.tensor_copy(out=dnr[:], in_=pctx[HD:HD + 1, :])
                        rc = smp.tile([1, TCH], F32, tag="rc")
                        with nc.allow_low_precision(reason="softmax denom"):
                            nc.vector.reciprocal_approx_fast(out=rc[:], in_=dnr[:])
                        rcb = smp.tile([1, TCH], BF16, tag="rcb")
                        nc.vector.tensor_copy(out=rcb[:], in_=rc[:])
                        # broadcast 1/denom over 64 partitions via a K=1
                        # ones-matmul (gpsimd partition_broadcast serializes
                        # against in-flight DMA queues - avoid it)
                        pb = pso.tile([128, TCH], F32, tag="pso",
                                      name=f"pb{icn}_{h}")
                        nc.tensor.matmul(pb[0:64, :], ONES[0:1, 0:64],
                                         rcb[0:1, :], start=True, stop=True)
                        bc = smp.tile([64, TCH], F32, tag="bc")
                        nc.scalar.copy(out=bc[:], in_=pb[0:64, :])
                        nc.vector.tensor_tensor(
                            out=CT[po:po + 64, cb, isl],
                            in0=pctx[0:HD, :], in1=bc[:],
                            op=OP.mult)

                    # output projection for this i-chunk's token blocks
                    for tb in range(icn * 4, icn * 4 + 4):
                        for on in range(2):
                            p = pso.tile([128, TCH], F32, tag="pso")
                            for cbk in range(2):
                                nc.tensor.matmul(
                                    p[:],
                                    CT[:, cbk, tb * 128:(tb + 1) * 128],
                                    WO[:, cbk, on * TCH:(on + 1) * TCH],
                                    start=(cbk == 0), stop=(cbk == 1))
                            ob_sb = ostp.tile([128, TCH], F32, tag="ost")
                            nc.vector.tensor_copy(out=ob_sb[:], in_=p[:])
                            nc.sync.dma_start(
                                out=out_d[tb * 128:(tb + 1) * 128,
                                          on * TCH:(on + 1) * TCH],
                                in_=ob_sb[:])

    nc.compile()
    return nc


def _get_nc(has_bias: bool):
    key = ("nc", has_bias)
    if key not in _CACHE:
        _CACHE[key] = _build(has_bias)
    return _CACHE[key]


def _maybe_wire_ntff_hook():
    try:
        import antenv.axon_hooks  # noqa: F401  already present
        return
    except ImportError:
        pass
    try:
        import sys, types
        import trn_agent_boot.trn_boot as boot
        hook = boot._ntff_profile_via_ctypes("/opt/axon/libaxon_pjrt.so")
        mod = types.ModuleType("antenv.axon_hooks")
        mod.get_axon_ntff_profile_hook = lambda: hook
        mod.set_axon_ntff_profile_hook = lambda h: None
        sys.modules["antenv.axon_hooks"] = mod
    except Exception:
        pass


def kernel(x, Wq, bq, Wk, bk, Wv, bv, Wo, bo, _trace=False):
    global LAST_EXEC_NS, LAST_RESULTS
    x = np.asarray(x, np.float32)
    Wq = np.asarray(Wq, np.float32); bq = np.asarray(bq, np.float32)
    Wk = np.asarray(Wk, np.float32); bk = np.asarray(bk, np.float32)
    Wv = np.asarray(Wv, np.float32); bv = np.asarray(bv, np.float32)
    Wo = np.asarray(Wo, np.float32); bo = np.asarray(bo, np.float32)

    has_bias = bool(np.any(bq) or np.any(bk) or np.any(bv))
    nc = _get_nc(has_bias)

    BFNP = ml_dtypes.bfloat16
    ones = np.ones((1, T), BFNP)
    onesc = np.ones((128, HPC), BFNP)
    xTs = [np.ascontiguousarray(x[b].T).astype(BFNP) for b in range(B)]

    in_maps = []
    for c in range(NCORES):
        b, tpr = divmod(c, TP)
        rows = slice(CS * tpr, CS * (tpr + 1))
        in_maps.append({
            "xT": xTs[b],
            "wqT": np.ascontiguousarray(Wq[rows, :].T).astype(BFNP),
            "wkT": np.ascontiguousarray(Wk[rows, :].T).astype(BFNP),
            "wvT": np.ascontiguousarray(Wv[rows, :].T).astype(BFNP),
            "woT": np.ascontiguousarray(Wo[:, rows].T).astype(BFNP),
            "bq": np.ascontiguousarray(bq[rows]).reshape(1, CS).astype(BFNP),
            "bk": np.ascontiguousarray(bk[rows]).reshape(1, CS).astype(BFNP),
            "bv": np.ascontiguousarray(bv[rows]).reshape(1, CS).astype(BFNP),
            "ones": ones,
            "onesc": onesc,
        })

    if _trace:
        _maybe_wire_ntff_hook()
    res = run_bass_kernel_spmd(nc, in_maps, core_ids=list(range(NCORES)),
                               trace=bool(_trace))
    LAST_EXEC_NS = res.exec_time_ns
    LAST_RESULTS = res

    out = np.empty((B, T, D), np.float32)
    for b in range(B):
        acc = res.results[TP * b]["out"].astype(np.float32)
        for tpr in range(1, TP):
            acc = acc + res.results[TP * b + tpr]["out"]
        out[b] = acc + bo[None, :]
    return out
